# revision 9
# baseline (speedup 1.0000x reference)
"""Trainium2 Bass kernel for the FERMI fairness-regularizer loss.

Math (see reference):
    h   = relu(X @ fc1_w.T + fc1_b)              [B, H]
    yh  = sigmoid(h @ fc2_w.T + fc2_b)[:, 0]     [B]
    out = LAM * (-sum(yh^2)*||W||_F^2 + 2*sum(yh * (S @ P @ W)) - B) / B

Strategy (pure data parallel over batch, 8 cores):
  - Host pre-transposes X so the contraction dim (D=784) lands on SBUF
    partitions (6 chunks of 128 + one 16-row tail, keeping DMAs at full
    128-partition port width) and casts X and fc1 weights to fp8-e4m3
    (fc1 scaled by 2**12 into e4m3's normal range; the sigmoid's input
    scale undoes it for free). PSUM accumulation stays fp32; HW-validated
    rel err ~1.5e-6 on the final scalar, which is dominated by the
    constant -B term, so unbiased fp8 rounding cancels over 131k samples.
  - mm1 runs in DoubleRow fp8 perf mode: 256 contraction rows per matmul,
    2 fp8 weights per PE cell (weight pair stride HP=112 is 16-aligned as
    the ISA requires).
  - |fc2_w| is folded into fc1 rows (relu(c*x) = c*relu(x) for c > 0), so
    the second matmul contracts with a {-1,0,+1} sign vector and no
    per-partition multiply is needed between the two matmuls.
  - Each core computes yh for its 16384 samples; the three batch sums
    (the "all-reduce" of the sharding hint) are done on host in float64.
  - ALL per-core device inputs live in ONE fp8 DRAM tensor ("blob"):
    the X stream in its per-unit packing, then the fc1 weights, the
    16-row X tail, the tail weights, and (bitcast) the bf16 sign vector
    and f32 biases. A single input cuts per-launch buffer-marshalling
    overhead vs six separate tensors.
  - yh is returned as bf16 (plenty for three batch-sum reductions that
    are diluted ~100x by the constant -B term) and stored in 2048-sample
    chunks as sigmoids complete, so the post-compute drain DMA is 4KB,
    not 16KB, on the single SBUF partition that holds yh.
  - Pipeline: X packed host-side per load-unit so every DMA is one long
    per-partition run; tapered units (512 head, 512->256 drain, 1024
    mid-stream); const loads ride the ACT HWDGE ring so the X stream
    owns the SP ring; sigmoid batched 1024-wide mid-stream, 512-wide
    near the drain end.
"""

import sys

try:
    import concourse  # noqa: F401
except ImportError:
    sys.path.insert(0, "/opt/trn_rl_repo")

import ml_dtypes
import numpy as np

import concourse.bass as bass  # noqa: F401  (Bass types used via bacc/tile)
import concourse.tile as tile
from concourse import bacc, mybir
from concourse.bass_utils import run_bass_kernel_spmd

# Problem constants (hardcoded per contract)
B, D, H = 131072, 784, 100
HP = 112                    # H padded to a 16-aligned DoubleRow weight stride
LAM = 0.1
# 4 cores, not 8: the axon terminal's per-launch service cost scales with
# core count (~2x from 4->8 cores, measured), while the per-core DMA time
# at 4 cores (~72us) still hides under it. Net: lower steady-state
# per-execution time than the 8-core split despite double the per-core work.
N_CORES = 4
BS = B // N_CORES           # 32768 samples per core
PD, ND = 128, 6             # 768 = 6 * 128 main contraction chunks
TD = D - PD * ND            # 16-row tail chunk
F_MM = 512                  # matmul moving free dim (one PSUM bank fp32)
F_BIG = 1024                # samples per DMA super-tile
N_SUPER = BS // F_BIG       # 16
N_MM = F_BIG // F_MM        # 2

_BF16 = ml_dtypes.bfloat16
_F8 = ml_dtypes.float8_e4m3     # == TRN float8e4 (max normal 240)
K_SCALE = 12                    # fc1 weights scaled by 2**12 into e4m3 range

# Column layout of the consolidated per-core input blob [PD, COLS] (fp8).
# Regions used by <128 rows simply leave the other rows unread.
C_XT = 0
L_XT = ND * BS                       # 98304: X stream, per-unit packed
C_AT = C_XT + L_XT                   # 98304: fc1 weights [128, 6*112]
L_AT = ND * HP                       # 672
C_XTL = C_AT + L_AT                  # 98976: X tail rows [16, BS]
L_XTL = BS
C_ATL = C_XTL + L_XTL                # 115360: tail weights [16, 100]
L_ATL = H
C_SGN = ((C_ATL + L_ATL + 7) // 8) * 8   # 115464: sign vec bf16 [100,1] = 2B
C_B12 = ((C_SGN + 2 + 7) // 8) * 8       # 115472: biases f32 [100,2] = 8B
COLS = ((C_B12 + 8 + 15) // 16) * 16     # 115488 (16B-aligned row stride)

F_ST = 2048                 # yh store chunk (bf16, 4KB from one partition)

_compiled_nc = None


def _unit_lens():
    head = [F_MM] * 4
    tail_t = [F_MM] * 7 + [F_MM // 2, F_MM // 2]
    mid = (BS - sum(head) - sum(tail_t)) // F_BIG
    lens = head + [F_BIG] * mid + tail_t
    assert sum(lens) == BS
    return lens


def _build_bass():
    """Per-core SPMD program. Identical on all 8 cores (no collectives)."""
    nc = bacc.Bacc("TRN2", target_bir_lowering=False, debug=False,
                   enable_partition_id=False)
    f32, bf16 = mybir.dt.float32, mybir.dt.bfloat16

    f8 = mybir.dt.float8e4
    blob = nc.dram_tensor("blob", [PD, COLS], f8, kind="ExternalInput")
    yh = nc.dram_tensor("yh", [1, BS], bf16, kind="ExternalOutput")

    with tile.TileContext(nc) as tc:
        with (
            tc.tile_pool(name="consts", bufs=1) as consts,
            tc.tile_pool(name="xpool", bufs=5) as xpool,
            tc.tile_pool(name="gpool", bufs=6) as gpool,
            tc.tile_pool(name="ypool", bufs=1) as ypool,
            tc.tile_pool(name="hpsum", bufs=4, space="PSUM") as hpsum,
            tc.tile_pool(name="ypsum", bufs=2, space="PSUM") as ypsum,
        ):
            # Load units: 1024-sample DMAs (~0.8 MiB) in the middle for low
            # per-DMA overhead; tapered smaller units at both ends so the
            # pipeline fills sooner and drains with less work after the last
            # byte. (off, len) pairs in samples over the whole shard.
            lens = _unit_lens()
            units, pos = [], 0
            for ln in lens:
                units.append((pos, ln))
                pos += ln

            def load_unit(off, ln):
                x_sb = xpool.tile([PD, ND, ln], f8, tag="x_sb")
                nc.sync.dma_start(
                    out=x_sb.rearrange("p n f -> p (n f)"),
                    in_=blob[:, ND * off:ND * (off + ln)])
                return x_sb

            # First big X DMA goes out before anything else on the SP HWDGE
            # ring; const loads ride the ACT ring so they overlap it.
            x_first = load_unit(*units[0])

            a_sb = consts.tile([PD, ND, HP], f8, tag="a_sb")
            nc.scalar.dma_start(out=a_sb.rearrange("p n h -> p (n h)"),
                                in_=blob[:, C_AT:C_AT + L_AT])
            xtl_sb = consts.tile([TD, BS], f8, tag="xtl_sb")
            nc.scalar.dma_start(out=xtl_sb[:],
                                in_=blob[0:TD, C_XTL:C_XTL + L_XTL])
            atl_sb = consts.tile([TD, H], f8, tag="atl_sb")
            nc.scalar.dma_start(out=atl_sb[:],
                                in_=blob[0:TD, C_ATL:C_ATL + L_ATL])
            sgn_sb = consts.tile([H, 1], bf16, tag="sgn_sb")
            nc.scalar.dma_start(out=sgn_sb[:],
                                in_=blob[0:H, C_SGN:C_SGN + 2].bitcast(bf16))
            b12_sb = consts.tile([H, 2], f32, tag="b12_sb")
            nc.scalar.dma_start(out=b12_sb[:],
                                in_=blob[0:H, C_B12:C_B12 + 8].bitcast(f32))
            b1_sb = b12_sb[:, 0:1]
            b2_sb = b12_sb[0:1, 1:2]

            yh_sb = ypool.tile([1, BS], bf16, tag="yh_sb")

            # Sigmoid batches: 1024-wide mid-stream (fewer ACT ops), per-tile
            # at the tapered end so the final chain after the last X byte is
            # short.
            F_SIG = 1024
            sig_groups, pos2 = [], 0
            for ln in lens:
                n_sub = max(1, ln // F_MM)
                for f in range(n_sub):
                    t_off = pos2 + f * F_MM
                    if t_off >= BS - F_BIG:
                        # 512-wide groups near the drain end — smaller ACT ops
                        # de-stack the final sigmoid serialization
                        if t_off % F_MM == 0:
                            sig_groups.append((t_off, t_off + F_MM))
                    elif t_off % F_SIG == 0:
                        sig_groups.append((t_off, t_off + F_SIG))
                pos2 += ln
            group_start = {s: (s, e) for s, e in sig_groups}
            group_end = {e: (s, e) for s, e in sig_groups}
            yp_big, yp_base = None, 0
            for iu, (u_off, u_len) in enumerate(units):
                x_sb = x_first if iu == 0 else load_unit(u_off, u_len)
                n_sub = max(1, u_len // F_MM)
                for f in range(n_sub):
                    off = u_off + f * F_MM
                    ln = min(F_MM, u_len)
                    hp_t = hpsum.tile([HP, F_MM], f32, tag="hp")
                    hp = hp_t[:, :ln]
                    for dc in range(ND // 2):
                        # DoubleRow: contract 256 d-rows (two 128-chunks) per
                        # matmul, 2 fp8 weights per PE cell. Weight pair
                        # stride HP=112 is 16-aligned as the ISA requires.
                        nc.tensor.matmul(
                            hp[:],
                            lhsT=a_sb[:, 2 * dc:2 * dc + 2, :],
                            rhs=x_sb[:, 2 * dc:2 * dc + 2,
                                     f * F_MM:f * F_MM + ln],
                            start=(dc == 0),
                            stop=False,
                            perf_mode=mybir.MatmulPerfMode.DoubleRow,
                        )
                    nc.tensor.matmul(
                        hp[:H, :], lhsT=atl_sb[:],
                        rhs=xtl_sb[:, off:off + ln],
                        start=False, stop=True,
                    )
                    # g = relu(hp + b1), cast to bf16 for the second matmul
                    g_t = gpool.tile([H, F_MM], bf16, tag="g")
                    g = g_t[:, :ln]
                    nc.vector.tensor_scalar(
                        out=g[:], in0=hp[:H, :],
                        scalar1=b1_sb[:], scalar2=0.0,
                        op0=mybir.AluOpType.add, op1=mybir.AluOpType.max,
                    )
                    if off in group_start:
                        gs, ge = group_start[off]
                        yp_big = ypsum.tile([1, ge - gs], f32, tag="yp")
                        yp_base = gs
                    sub = off - yp_base
                    nc.tensor.matmul(yp_big[:, sub:sub + ln], lhsT=sgn_sb[:],
                                     rhs=g[:], start=True, stop=True)
                    if off + ln in group_end:
                        gs, ge = group_end[off + ln]
                        nc.scalar.activation(
                            out=yh_sb[:, gs:ge], in_=yp_big[:],
                            func=mybir.ActivationFunctionType.Sigmoid,
                            bias=b2_sb[:], scale=float(2.0 ** -K_SCALE),
                        )
                        # Stream completed yh chunks out as we go; keeps the
                        # post-compute drain to one 4KB store. The mid-stream
                        # stores ride the ACT ring (idle between sigmoids;
                        # sim-checked against Pool/per-group variants — fewer
                        # tail DMAs wins because HWDGE descriptor-gen
                        # serializes globally at ~500ns per DMA).
                        q = off + ln
                        if q % F_ST == 0 and q < BS:
                            nc.scalar.dma_start(
                                out=yh[:, q - F_ST:q],
                                in_=yh_sb[:, q - F_ST:q])
            # X stream is finished by now — the SP HWDGE ring is idle, so
            # the final chunk rides it instead of queueing behind the ACT
            # ring's last sigmoid dispatch.
            nc.sync.dma_start(out=yh[:, BS - F_ST:],
                              in_=yh_sb[:, BS - F_ST:])
    nc.compile()
    return nc


def _get_nc():
    global _compiled_nc
    if _compiled_nc is None:
        _compiled_nc = _build_bass()
    return _compiled_nc


def _pack_inputs(X, S, W, fc1_w, fc1_b, fc2_w, fc2_b, P):
    """Host-side prep: fold |fc2_w| into fc1, transpose/pack/cast X, and
    concatenate everything into one per-core fp8 blob."""
    c = np.asarray(fc2_w, np.float32)[0]                  # [H]
    absc = np.abs(c)
    sgn_v = np.sign(c).astype(_BF16).reshape(H, 1)
    A = np.asarray(fc1_w, np.float32) * absc[:, None]     # [H, D]
    AT = np.ascontiguousarray(A.T) * np.float32(2.0 ** K_SCALE)
    AT = np.clip(AT, -240.0, 240.0).astype(_F8)           # [D, H]
    ATp = np.zeros((D, HP), _F8)
    ATp[:, :H] = AT
    # [p, n, h]: per-partition contiguous weight DMA
    a_t = np.ascontiguousarray(
        ATp[:PD * ND].reshape(ND, PD, HP).transpose(1, 0, 2))
    a_tl = AT[PD * ND:]                                   # [TD, H]
    b12_v = np.zeros((H, 2), np.float32)
    b12_v[:, 0] = (np.asarray(fc1_b, np.float32) * absc
                   * np.float32(2.0 ** K_SCALE))
    b12_v[0, 1] = np.float32(np.asarray(fc2_b, np.float32).reshape(-1)[0])

    Xb = np.clip(np.asarray(X, np.float32), -240.0, 240.0).astype(_F8)
    lens = _unit_lens()
    in_maps = []
    for core in range(N_CORES):
        xs = Xb[core * BS:(core + 1) * BS]                # [BS, D]
        blob = np.zeros((PD, COLS), _F8)
        # X stream, per-unit-contiguous: blob[p, ND*off + n*ln + col]
        #   = X[off + col, n*128 + p]
        off = 0
        for ln in lens:
            blk = xs[off:off + ln, :PD * ND].reshape(ln, ND, PD)
            blob[:, ND * off:ND * (off + ln)] = (
                blk.transpose(2, 1, 0).reshape(PD, ND * ln))
            off += ln
        blob[:, C_AT:C_AT + L_AT] = a_t.reshape(PD, L_AT)
        blob[:TD, C_XTL:C_XTL + L_XTL] = xs[:, PD * ND:].T
        blob[:TD, C_ATL:C_ATL + L_ATL] = a_tl
        bview = blob.view(np.uint8)
        bview[:H, C_SGN:C_SGN + 2] = sgn_v.view(np.uint8)
        bview[:H, C_B12:C_B12 + 8] = b12_v.view(np.uint8)
        in_maps.append({"blob": blob})
    return in_maps


def kernel(X, S, W, fc1_w, fc1_b, fc2_w, fc2_b, P):
    nc = _get_nc()
    in_maps = _pack_inputs(X, S, W, fc1_w, fc1_b, fc2_w, fc2_b, P)
    res = run_bass_kernel_spmd(nc, in_maps, core_ids=list(range(N_CORES)))
    yh = np.concatenate([r["yh"][0] for r in res.results]).astype(np.float64)

    S64 = np.asarray(S, np.float64)
    W64 = np.asarray(W, np.float64)
    PW = np.asarray(P, np.float64) @ W64                  # [2, 1]
    tr_wwt = float(np.sum(W64 * W64))
    summation = (-float(yh @ yh) * tr_wwt
                 + 2.0 * float((yh @ S64) @ PW[:, 0])
                 - B)
    return np.float32(LAM * summation / B)


# revision 10
# speedup vs baseline: 1.3902x; 1.3902x over previous
"""Trainium2 Bass kernel for the FERMI fairness-regularizer loss.

Math (see reference):
    h   = relu(X @ fc1_w.T + fc1_b)              [B, H]
    yh  = sigmoid(h @ fc2_w.T + fc2_b)[:, 0]     [B]
    out = LAM * (-sum(yh^2)*||W||_F^2 + 2*sum(yh * (S @ P @ W)) - B) / B

Strategy (pure data parallel over batch, 8 cores):
  - Host pre-transposes X so the contraction dim (D=784) lands on SBUF
    partitions (6 chunks of 128 + one 16-row tail, keeping DMAs at full
    128-partition port width) and casts X and fc1 weights to fp8-e4m3
    (fc1 scaled by 2**12 into e4m3's normal range; the sigmoid's input
    scale undoes it for free). PSUM accumulation stays fp32; HW-validated
    rel err ~1.5e-6 on the final scalar, which is dominated by the
    constant -B term, so unbiased fp8 rounding cancels over 131k samples.
  - mm1 runs in DoubleRow fp8 perf mode: 256 contraction rows per matmul,
    2 fp8 weights per PE cell (weight pair stride HP=112 is 16-aligned as
    the ISA requires).
  - |fc2_w| is folded into fc1 rows (relu(c*x) = c*relu(x) for c > 0), so
    the second matmul contracts with a {-1,0,+1} sign vector and no
    per-partition multiply is needed between the two matmuls.
  - Each core computes yh for its 16384 samples; the three batch sums
    (the "all-reduce" of the sharding hint) are done on host in float64.
  - ALL per-core device inputs live in ONE fp8 DRAM tensor ("blob"):
    the X stream in its per-unit packing, then the fc1 weights, the
    16-row X tail, the tail weights, and (bitcast) the bf16 sign vector
    and f32 biases. A single input cuts per-launch buffer-marshalling
    overhead vs six separate tensors.
  - yh is returned as bf16 (plenty for three batch-sum reductions that
    are diluted ~100x by the constant -B term) and stored in 2048-sample
    chunks as sigmoids complete, so the post-compute drain DMA is 4KB,
    not 16KB, on the single SBUF partition that holds yh.
  - Pipeline: X packed host-side per load-unit so every DMA is one long
    per-partition run; tapered units (512 head, 512->256 drain, 1024
    mid-stream); const loads ride the ACT HWDGE ring so the X stream
    owns the SP ring; sigmoid batched 1024-wide mid-stream, 512-wide
    near the drain end.
"""

import sys

try:
    import concourse  # noqa: F401
except ImportError:
    sys.path.insert(0, "/opt/trn_rl_repo")

import ml_dtypes
import numpy as np

import concourse.bass as bass  # noqa: F401  (Bass types used via bacc/tile)
import concourse.tile as tile
from concourse import bacc, mybir
from concourse.bass_utils import run_bass_kernel_spmd

# Problem constants (hardcoded per contract)
B, D, H = 131072, 784, 100
HP = 112                    # H padded to a 16-aligned DoubleRow weight stride
LAM = 0.1
N_CORES = 8
BS = B // N_CORES           # 16384 samples per core
PD, ND = 128, 6             # 768 = 6 * 128 main contraction chunks
TD = D - PD * ND            # 16-row tail chunk
F_MM = 512                  # matmul moving free dim (one PSUM bank fp32)
F_BIG = 1024                # samples per DMA super-tile
N_SUPER = BS // F_BIG       # 16
N_MM = F_BIG // F_MM        # 2

_BF16 = ml_dtypes.bfloat16
_F8 = ml_dtypes.float8_e4m3     # == TRN float8e4 (max normal 240)
K_SCALE = 12                    # fc1 weights scaled by 2**12 into e4m3 range

# Column layout of the consolidated per-core input blob [PD, COLS] (fp8).
# Regions used by <128 rows simply leave the other rows unread.
C_XT = 0
L_XT = ND * BS                       # 98304: X stream, per-unit packed
C_AT = C_XT + L_XT                   # 98304: fc1 weights [128, 6*112]
L_AT = ND * HP                       # 672
C_XTL = C_AT + L_AT                  # 98976: X tail rows [16, BS]
L_XTL = BS
C_ATL = C_XTL + L_XTL                # 115360: tail weights [16, 100]
L_ATL = H
C_SGN = ((C_ATL + L_ATL + 7) // 8) * 8   # 115464: sign vec bf16 [100,1] = 2B
C_B12 = ((C_SGN + 2 + 7) // 8) * 8       # 115472: biases f32 [100,2] = 8B
COLS = ((C_B12 + 8 + 15) // 16) * 16     # 115488 (16B-aligned row stride)

F_ST = 2048                 # yh store chunk (bf16, 4KB from one partition)

_compiled_nc = None


def _unit_lens():
    head = [F_MM] * 4
    tail_t = [F_MM] * 7 + [F_MM // 2, F_MM // 2]
    mid = (BS - sum(head) - sum(tail_t)) // F_BIG
    lens = head + [F_BIG] * mid + tail_t
    assert sum(lens) == BS
    return lens


def _build_bass():
    """Per-core SPMD program. Identical on all 8 cores (no collectives)."""
    nc = bacc.Bacc("TRN2", target_bir_lowering=False, debug=False,
                   enable_partition_id=False)
    f32, bf16 = mybir.dt.float32, mybir.dt.bfloat16

    f8 = mybir.dt.float8e4
    blob = nc.dram_tensor("blob", [PD, COLS], f8, kind="ExternalInput")
    yh = nc.dram_tensor("yh", [1, BS], bf16, kind="ExternalOutput")

    with tile.TileContext(nc) as tc:
        with (
            tc.tile_pool(name="consts", bufs=1) as consts,
            tc.tile_pool(name="xpool", bufs=5) as xpool,
            tc.tile_pool(name="gpool", bufs=6) as gpool,
            tc.tile_pool(name="ypool", bufs=1) as ypool,
            tc.tile_pool(name="hpsum", bufs=4, space="PSUM") as hpsum,
            tc.tile_pool(name="ypsum", bufs=2, space="PSUM") as ypsum,
        ):
            # Load units: 1024-sample DMAs (~0.8 MiB) in the middle for low
            # per-DMA overhead; tapered smaller units at both ends so the
            # pipeline fills sooner and drains with less work after the last
            # byte. (off, len) pairs in samples over the whole shard.
            lens = _unit_lens()
            units, pos = [], 0
            for ln in lens:
                units.append((pos, ln))
                pos += ln

            def load_unit(off, ln):
                x_sb = xpool.tile([PD, ND, ln], f8, tag="x_sb")
                nc.sync.dma_start(
                    out=x_sb.rearrange("p n f -> p (n f)"),
                    in_=blob[:, ND * off:ND * (off + ln)])
                return x_sb

            # First big X DMA goes out before anything else on the SP HWDGE
            # ring; const loads ride the ACT ring so they overlap it.
            x_first = load_unit(*units[0])

            a_sb = consts.tile([PD, ND, HP], f8, tag="a_sb")
            nc.scalar.dma_start(out=a_sb.rearrange("p n h -> p (n h)"),
                                in_=blob[:, C_AT:C_AT + L_AT])
            xtl_sb = consts.tile([TD, BS], f8, tag="xtl_sb")
            nc.scalar.dma_start(out=xtl_sb[:],
                                in_=blob[0:TD, C_XTL:C_XTL + L_XTL])
            atl_sb = consts.tile([TD, H], f8, tag="atl_sb")
            nc.scalar.dma_start(out=atl_sb[:],
                                in_=blob[0:TD, C_ATL:C_ATL + L_ATL])
            sgn_sb = consts.tile([H, 1], bf16, tag="sgn_sb")
            nc.scalar.dma_start(out=sgn_sb[:],
                                in_=blob[0:H, C_SGN:C_SGN + 2].bitcast(bf16))
            b12_sb = consts.tile([H, 2], f32, tag="b12_sb")
            nc.scalar.dma_start(out=b12_sb[:],
                                in_=blob[0:H, C_B12:C_B12 + 8].bitcast(f32))
            b1_sb = b12_sb[:, 0:1]
            b2_sb = b12_sb[0:1, 1:2]

            yh_sb = ypool.tile([1, BS], bf16, tag="yh_sb")

            # Sigmoid batches: 1024-wide mid-stream (fewer ACT ops), per-tile
            # at the tapered end so the final chain after the last X byte is
            # short.
            F_SIG = 1024
            sig_groups, pos2 = [], 0
            for ln in lens:
                n_sub = max(1, ln // F_MM)
                for f in range(n_sub):
                    t_off = pos2 + f * F_MM
                    if t_off >= BS - F_BIG:
                        # 512-wide groups near the drain end — smaller ACT ops
                        # de-stack the final sigmoid serialization
                        if t_off % F_MM == 0:
                            sig_groups.append((t_off, t_off + F_MM))
                    elif t_off % F_SIG == 0:
                        sig_groups.append((t_off, t_off + F_SIG))
                pos2 += ln
            group_start = {s: (s, e) for s, e in sig_groups}
            group_end = {e: (s, e) for s, e in sig_groups}
            yp_big, yp_base = None, 0
            for iu, (u_off, u_len) in enumerate(units):
                x_sb = x_first if iu == 0 else load_unit(u_off, u_len)
                n_sub = max(1, u_len // F_MM)
                for f in range(n_sub):
                    off = u_off + f * F_MM
                    ln = min(F_MM, u_len)
                    hp_t = hpsum.tile([HP, F_MM], f32, tag="hp")
                    hp = hp_t[:, :ln]
                    for dc in range(ND // 2):
                        # DoubleRow: contract 256 d-rows (two 128-chunks) per
                        # matmul, 2 fp8 weights per PE cell. Weight pair
                        # stride HP=112 is 16-aligned as the ISA requires.
                        nc.tensor.matmul(
                            hp[:],
                            lhsT=a_sb[:, 2 * dc:2 * dc + 2, :],
                            rhs=x_sb[:, 2 * dc:2 * dc + 2,
                                     f * F_MM:f * F_MM + ln],
                            start=(dc == 0),
                            stop=False,
                            perf_mode=mybir.MatmulPerfMode.DoubleRow,
                        )
                    nc.tensor.matmul(
                        hp[:H, :], lhsT=atl_sb[:],
                        rhs=xtl_sb[:, off:off + ln],
                        start=False, stop=True,
                    )
                    # g = relu(hp + b1), cast to bf16 for the second matmul
                    g_t = gpool.tile([H, F_MM], bf16, tag="g")
                    g = g_t[:, :ln]
                    nc.vector.tensor_scalar(
                        out=g[:], in0=hp[:H, :],
                        scalar1=b1_sb[:], scalar2=0.0,
                        op0=mybir.AluOpType.add, op1=mybir.AluOpType.max,
                    )
                    if off in group_start:
                        gs, ge = group_start[off]
                        yp_big = ypsum.tile([1, ge - gs], f32, tag="yp")
                        yp_base = gs
                    sub = off - yp_base
                    nc.tensor.matmul(yp_big[:, sub:sub + ln], lhsT=sgn_sb[:],
                                     rhs=g[:], start=True, stop=True)
                    if off + ln in group_end:
                        gs, ge = group_end[off + ln]
                        nc.scalar.activation(
                            out=yh_sb[:, gs:ge], in_=yp_big[:],
                            func=mybir.ActivationFunctionType.Sigmoid,
                            bias=b2_sb[:], scale=float(2.0 ** -K_SCALE),
                        )
                        # Stream completed yh chunks out as we go; keeps the
                        # post-compute drain to one 4KB store. The mid-stream
                        # stores ride the ACT ring (idle between sigmoids;
                        # sim-checked against Pool/per-group variants — fewer
                        # tail DMAs wins because HWDGE descriptor-gen
                        # serializes globally at ~500ns per DMA).
                        q = off + ln
                        if q % F_ST == 0 and q < BS:
                            nc.scalar.dma_start(
                                out=yh[:, q - F_ST:q],
                                in_=yh_sb[:, q - F_ST:q])
            # X stream is finished by now — the SP HWDGE ring is idle, so
            # the final chunk rides it instead of queueing behind the ACT
            # ring's last sigmoid dispatch.
            nc.sync.dma_start(out=yh[:, BS - F_ST:],
                              in_=yh_sb[:, BS - F_ST:])
    nc.compile()
    return nc


def _get_nc():
    global _compiled_nc
    if _compiled_nc is None:
        _compiled_nc = _build_bass()
    return _compiled_nc


def _pack_inputs(X, S, W, fc1_w, fc1_b, fc2_w, fc2_b, P):
    """Host-side prep: fold |fc2_w| into fc1, transpose/pack/cast X, and
    concatenate everything into one per-core fp8 blob."""
    c = np.asarray(fc2_w, np.float32)[0]                  # [H]
    absc = np.abs(c)
    sgn_v = np.sign(c).astype(_BF16).reshape(H, 1)
    A = np.asarray(fc1_w, np.float32) * absc[:, None]     # [H, D]
    AT = np.ascontiguousarray(A.T) * np.float32(2.0 ** K_SCALE)
    AT = np.clip(AT, -240.0, 240.0).astype(_F8)           # [D, H]
    ATp = np.zeros((D, HP), _F8)
    ATp[:, :H] = AT
    # [p, n, h]: per-partition contiguous weight DMA
    a_t = np.ascontiguousarray(
        ATp[:PD * ND].reshape(ND, PD, HP).transpose(1, 0, 2))
    a_tl = AT[PD * ND:]                                   # [TD, H]
    b12_v = np.zeros((H, 2), np.float32)
    b12_v[:, 0] = (np.asarray(fc1_b, np.float32) * absc
                   * np.float32(2.0 ** K_SCALE))
    b12_v[0, 1] = np.float32(np.asarray(fc2_b, np.float32).reshape(-1)[0])

    Xb = np.clip(np.asarray(X, np.float32), -240.0, 240.0).astype(_F8)
    lens = _unit_lens()
    in_maps = []
    for core in range(N_CORES):
        xs = Xb[core * BS:(core + 1) * BS]                # [BS, D]
        blob = np.zeros((PD, COLS), _F8)
        # X stream, per-unit-contiguous: blob[p, ND*off + n*ln + col]
        #   = X[off + col, n*128 + p]
        off = 0
        for ln in lens:
            blk = xs[off:off + ln, :PD * ND].reshape(ln, ND, PD)
            blob[:, ND * off:ND * (off + ln)] = (
                blk.transpose(2, 1, 0).reshape(PD, ND * ln))
            off += ln
        blob[:, C_AT:C_AT + L_AT] = a_t.reshape(PD, L_AT)
        blob[:TD, C_XTL:C_XTL + L_XTL] = xs[:, PD * ND:].T
        blob[:TD, C_ATL:C_ATL + L_ATL] = a_tl
        bview = blob.view(np.uint8)
        bview[:H, C_SGN:C_SGN + 2] = sgn_v.view(np.uint8)
        bview[:H, C_B12:C_B12 + 8] = b12_v.view(np.uint8)
        in_maps.append({"blob": blob})
    return in_maps


def kernel(X, S, W, fc1_w, fc1_b, fc2_w, fc2_b, P):
    nc = _get_nc()
    in_maps = _pack_inputs(X, S, W, fc1_w, fc1_b, fc2_w, fc2_b, P)
    res = run_bass_kernel_spmd(nc, in_maps, core_ids=list(range(N_CORES)))
    yh = np.concatenate([r["yh"][0] for r in res.results]).astype(np.float64)

    S64 = np.asarray(S, np.float64)
    W64 = np.asarray(W, np.float64)
    PW = np.asarray(P, np.float64) @ W64                  # [2, 1]
    tr_wwt = float(np.sum(W64 * W64))
    summation = (-float(yh @ yh) * tr_wwt
                 + 2.0 * float((yh @ S64) @ PW[:, 0])
                 - B)
    return np.float32(LAM * summation / B)
